# revision 1
# baseline (speedup 1.0000x reference)
"""Trainium2 Bass kernel for a 2-layer GAT + link predictor (nn_GAT).

Strategy (8 NeuronCores, SPMD single program):
  - Nodes are permuted and assigned to (core, rank) slots so every 128-rank
    block carries a near-equal number of incoming edges (load balance AND
    uniform program structure across cores).
  - Per layer:
      phase A (dense):  h = x @ W via PE; h rows written to DRAM.
      phase B (edges, sharded by dst): dma_gather h[src] rows; per-edge
        attention logits via on-chip dots + fp8 one-hot matmuls; softmax
        without max-subtraction (mathematically identical); weighted
        aggregation + denominator accumulate in PSUM via fp8 one-hot
        matmuls.  Per-core rank rows of s_dst are fetched with an indexed
        gather (per-core index inputs keep the program core-independent).
  - AllGather (DRAM collective) of layer outputs between layers.
  - Link predictor sharded over queries, 4-way grouped by (e0,e1) int16
    halves.
All index-derived data (permutations, one-hot tiles, gather indices) is
precomputed on the host from edge_index/edges; weights-derived scalars are
folded on the host (BN fold, W@a_dst).
"""
import heapq

import numpy as np
import ml_dtypes

from concourse import bass, bacc, tile
from concourse.bass_utils import run_bass_kernel_spmd

mybir = bass.mybir
AF = mybir.ActivationFunctionType
OP = mybir.AluOpType

F32 = mybir.dt.float32
BF16 = mybir.dt.bfloat16
F16 = mybir.dt.float16
F8 = mybir.dt.float8e4
I16 = mybir.dt.int16

N, C, E, Q = 50000, 128, 800000, 200000
NCORES = 8
NSH = N // NCORES            # 6250 nodes/core
RB = 128                     # ranks per block
NRB = (NSH + RB - 1) // RB   # 49 blocks/core
HALF = N // 2
RBS_PER_CHUNK = 3
BN_EPS = 1e-5

np_bf16 = ml_dtypes.bfloat16
np_f8 = ml_dtypes.float8_e4m3


# ----------------------------------------------------------------- host prep

def _build_perm(dst):
    """node -> (core, rank) assignment balancing edges per rank-block."""
    deg = np.bincount(dst, minlength=N)
    order = np.argsort(-deg, kind="stable")
    nblocks = NCORES * NRB
    caps = np.array([min(RB, NSH - rb * RB)
                     for k in range(NCORES) for rb in range(NRB)], np.int64)
    cnt = np.zeros(nblocks, np.int64)
    fill = [[] for _ in range(nblocks)]
    heap = [(0, b) for b in range(nblocks)]
    heapq.heapify(heap)
    for n in order:
        while True:
            load, b = heapq.heappop(heap)
            if cnt[b] < caps[b]:
                break
        fill[b].append(n)
        cnt[b] += 1
        if cnt[b] < caps[b]:
            heapq.heappush(heap, (load + int(deg[n]), b))
    perm = np.concatenate([np.array(fill[b], np.int64) for b in range(nblocks)])
    iperm = np.zeros(N, np.int64)
    iperm[perm] = np.arange(N)
    return perm, iperm


def _wrap_idx(idx):
    """[n] int -> [128, n//16] int16 (wrapped in 16 partitions, replicated)."""
    a = np.asarray(idx, np.int16).reshape(-1, 16).T
    return np.ascontiguousarray(np.tile(a, (8, 1)))


class _Tile:
    __slots__ = ("rb", "rbslot", "first", "last", "gslot")

    def __init__(self, rb, rbslot, first, last, gslot):
        self.rb, self.rbslot = rb, rbslot
        self.first, self.last = first, last
        self.gslot = gslot


class _Chunk:
    __slots__ = ("rbs", "tiles", "nA", "nB", "gcolA", "gcolB", "tilebase",
                 "rbcol")

    def __init__(self):
        self.rbs, self.tiles = [], []


def _prep(inputs):
    """All host-side index preprocessing. Returns a structure dict."""
    src = np.asarray(inputs["edge_index"][0], np.int64)
    dst = np.asarray(inputs["edge_index"][1], np.int64)
    perm, iperm = _build_perm(dst)
    src_p, dst_p = iperm[src], iperm[dst]

    dst_core = dst_p // NSH
    percore = []
    ntile_rbh = np.zeros((NRB, 2), np.int64)
    for k in range(NCORES):
        m = dst_core == k
        s, d = src_p[m], dst_p[m]
        rank = d - k * NSH
        rb = rank // RB
        half = (s >= HALF).astype(np.int64)
        percore.append((s, rank, rb, half))
        for r in range(NRB):
            c0 = int(((rb == r) & (half == 0)).sum())
            c1 = int(((rb == r) & (half == 1)).sum())
            ntile_rbh[r, 0] = max(ntile_rbh[r, 0], (c0 + 127) // 128)
            ntile_rbh[r, 1] = max(ntile_rbh[r, 1], (c1 + 127) // 128)
    ntile_rbh = np.maximum(ntile_rbh, 1)
    T = int(ntile_rbh.sum())

    # chunk structure (same for all cores)
    chunks = []
    tilebase = 0
    for c0 in range(0, NRB, RBS_PER_CHUNK):
        ch = _Chunk()
        ch.rbs = list(range(c0, min(c0 + RBS_PER_CHUNK, NRB)))
        ch.tilebase = tilebase
        gslot = 0
        for h in (0, 1):
            for rbslot, r in enumerate(ch.rbs):
                nt = int(ntile_rbh[r, h])
                for j in range(nt):
                    first = (h == 0 and j == 0)
                    last = (h == 1 and j == nt - 1)
                    ch.tiles.append(_Tile(r, rbslot, first, last, gslot))
                    gslot += 1
            if h == 0:
                ch.nA = gslot
        ch.nB = gslot - ch.nA
        tilebase += gslot
        chunks.append(ch)
    assert tilebase == T
    NTmax = max(ch.nA + ch.nB for ch in chunks)

    # gidx column ranges per (chunk, half): laid consecutively
    col = 0
    for ch in chunks:
        ch.gcolA = col
        col += ch.nA * 8
        ch.gcolB = col
        col += ch.nB * 8
    gcols = col
    # rb gather idx columns: 8 per rb, chunk slice = rbs
    for ch in chunks:
        ch.rbcol = ch.rbs[0] * 8

    # per-core edge data
    core_edge = []
    for k in range(NCORES):
        s, rank, rb, half = percore[k]
        gidx = np.zeros((128, gcols), np.int16)
        oh = np.zeros((128, T * 128), np_f8)
        ohT = np.zeros((128, T * 128), np_f8)
        for ch in chunks:
            for h in (0, 1):
                colbase = ch.gcolA if h == 0 else ch.gcolB
                slot0 = 0 if h == 0 else ch.nA
                nslots = ch.nA if h == 0 else ch.nB
                idxs = np.zeros(nslots * 128, np.int64)
                pos = 0
                for r in ch.rbs:
                    m2 = (rb == r) & (half == h)
                    ss = s[m2] - h * HALF
                    rk = rank[m2] - r * RB
                    nt = int(ntile_rbh[r, h])
                    idxs[pos:pos + len(ss)] = ss
                    # one-hot data, global tile index
                    gt0 = ch.tilebase + slot0 + pos // 128
                    e_in = np.arange(len(ss))
                    gpos = pos + e_in           # slot-local position
                    tt = ch.tilebase + slot0 + gpos // 128
                    pp = gpos % 128
                    oh[pp, tt * 128 + rk] = 1.0
                    ohT[rk, tt * 128 + pp] = 1.0
                    pos += nt * 128
                gidx[:, colbase:colbase + nslots * 8] = _wrap_idx(idxs)
        # rb row gathers: rank rows of this core, half-local with dummy 0
        base_rows = k * NSH + np.arange(NRB * RB)
        rbr = np.minimum(base_rows, (k + 1) * NSH - 1)  # clamp pad rows
        if k < 4:
            ra, rbb = rbr, np.zeros_like(rbr)
        else:
            ra, rbb = np.zeros_like(rbr), rbr - HALF
        core_edge.append(dict(
            gidx=gidx, oh=oh, ohT=ohT,
            rbidxA=_wrap_idx(ra), rbidxB=_wrap_idx(rbb),
        ))

    # ---- queries
    e0 = iperm[np.asarray(inputs["edges"][0], np.int64)]
    e1 = iperm[np.asarray(inputs["edges"][1], np.int64)]
    QSH = Q // NCORES
    qgrp_tiles = np.zeros(4, np.int64)
    qcore = []
    for k in range(NCORES):
        sl = slice(k * QSH, (k + 1) * QSH)
        a, b = e0[sl], e1[sl]
        g = (a >= HALF).astype(np.int64) * 2 + (b >= HALF).astype(np.int64)
        qcore.append((a, b, g))
        for gi in range(4):
            cnt = int((g == gi).sum())
            qgrp_tiles[gi] = max(qgrp_tiles[gi], (cnt + 127) // 128)
    QTT = int(qgrp_tiles.sum())
    # subchunks of <=32 tiles per group
    qchunks = []   # (group, tile0_in_out, ntiles)
    tpos = 0
    for gi in range(4):
        nt = int(qgrp_tiles[gi])
        j = 0
        while j < nt:
            step = min(32, nt - j)
            qchunks.append((gi, tpos + j, step))
            j += step
        tpos += nt

    core_q = []
    for k in range(NCORES):
        a, b, g = qcore[k]
        qi0 = np.zeros(QTT * 128, np.int64)
        qi1 = np.zeros(QTT * 128, np.int64)
        qmap = np.full(QTT * 128, -1, np.int64)
        tpos = 0
        for gi in range(4):
            m = g == gi
            cnt = int(m.sum())
            qi0[tpos:tpos + cnt] = a[m] - (gi >> 1) * HALF
            qi1[tpos:tpos + cnt] = b[m] - (gi & 1) * HALF
            qmap[tpos:tpos + cnt] = np.nonzero(m)[0] + k * QSH
            tpos += int(qgrp_tiles[gi]) * 128
        core_q.append(dict(qidx0=_wrap_idx(qi0), qidx1=_wrap_idx(qi1),
                           qmap=qmap))

    return dict(perm=perm, iperm=iperm, chunks=chunks, ntile_rbh=ntile_rbh,
                T=T, NTmax=NTmax, gcols=gcols, core_edge=core_edge,
                qgrp_tiles=qgrp_tiles, qchunks=qchunks, QTT=QTT,
                core_q=core_q)


def _rep(v):
    """[C] -> [128, C] replicated fp32."""
    return np.ascontiguousarray(np.broadcast_to(
        np.asarray(v, np.float32)[None, :], (128, C)))


def _rep16(v):
    """[C] -> [128, C] replicated fp16."""
    return np.ascontiguousarray(np.broadcast_to(
        np.asarray(v, np.float16)[None, :], (128, C)))


# ------------------------------------------------------------ program build

TUNE = dict(pb_bufs=3, dve_mod=1, dve_thr=0)


def _build_program(S, fast1, bp2val, upto=None):
    """Build the SPMD Bass program. S is the _prep structure.
    upto: truncate after the named stage (HW bisection; output stays zero).

    Sharded design: each core computes h (and s_dst) only for its own NSH
    nodes; fp16 h shards are AllGathered into a Shared h_full that the
    edge-phase gathers read. s_dst per rank block comes straight out of the
    phase-A matmul (W_aug = [W | W@a_dst]) and stays in SBUF."""
    nc = bacc.Bacc("TRN2", target_bir_lowering=False, debug=False,
                   num_devices=NCORES, num_swdge_queues=2)
    T, NTmax, QTT = S["T"], S["NTmax"], S["QTT"]
    chunks, qchunks = S["chunks"], S["qchunks"]

    def din(name, shape, dt):
        return nc.dram_tensor(name, shape, dt, kind="ExternalInput")

    # inputs
    embTk = din("embTk", [128, NSH], F32)        # per-core node slice
    W1aug = din("W1aug", [128, C + 1], F32)      # [W1 | W1@a_dst1]
    W2aug = din("W2aug", [128, C + 1], F32)
    a1rep = din("a1rep", [128, C], F16)
    a2rep = din("a2rep", [128, C], F16)
    Arep = din("Arep", [128, C], F32)
    C1rep = din("C1rep", [128, C], F32)
    b2rep = din("b2rep", [128, C], F32)
    Wp1bf = din("Wp1bf", [128, C], BF16)
    wp2bf = din("wp2bf", [128, 1], BF16)
    bp1bf = din("bp1bf", [1, C], BF16)
    ident = din("ident", [128, 128], F32)
    ohA = din("ohA", [128, T * 128], F8)
    ohTA = din("ohTA", [128, T * 128], F8)
    gidx = din("gidx", [128, S["gcols"]], I16)
    qidx0 = din("qidx0", [128, QTT * 8], I16)
    qidx1 = din("qidx1", [128, QTT * 8], I16)

    out_q = nc.dram_tensor("out_q", [1, QTT * 128], F32,
                           kind="ExternalOutput")

    # internal DRAM (h tables fp16: halves gather traffic + 2x DVE rate)
    h_sh = nc.dram_tensor("h_sh", [NSH, C], F16)
    h_full1 = nc.dram_tensor("h_full1", [N, C], F16, addr_space="Shared")
    h_full2 = nc.dram_tensor("h_full2", [N, C], F16, addr_space="Shared")
    x1T_sh = nc.dram_tensor("x1T_sh", [128, NSH], F32)
    x2_sh = nc.dram_tensor("x2_sh", [NSH, C], F16)
    x2_full = nc.dram_tensor("x2_full", [N, C], F16, addr_space="Shared")

    with tile.TileContext(nc) as tc:
        with tc.tile_pool(name="const", bufs=1) as cpool:
            def load_const(t, w=C, dt=F32):
                sb = cpool.tile([128, w], dt, tag=t.name)
                nc.sync.dma_start(sb[:], t[:])
                return sb

            W1_sb = load_const(W1aug, w=C + 1)
            W2_sb = load_const(W2aug, w=C + 1)
            a1_sb = load_const(a1rep, dt=F16)
            a2_sb = load_const(a2rep, dt=F16)
            b2_sb = load_const(b2rep)
            Wp1_sb = load_const(Wp1bf, dt=BF16)
            wp2_sb = load_const(wp2bf, w=1, dt=BF16)
            id_sb = load_const(ident, 128)
            bp1_sb = cpool.tile([1, C], BF16, tag="bp1bf")
            nc.sync.dma_start(bp1_sb[:], bp1bf[:])
            ones_sb = cpool.tile([1, 512], BF16, tag="ones")
            nc.gpsimd.memset(ones_sb[:], 1.0)
            if not fast1:
                A_sb = load_const(Arep)
                C1_sb = load_const(C1rep)
            gidx_sb = cpool.tile([128, S["gcols"]], I16)
            nc.sync.dma_start(gidx_sb[:], gidx[:])
            q0_sb = cpool.tile([128, QTT * 8], I16)
            nc.sync.dma_start(q0_sb[:], qidx0[:])
            q1_sb = cpool.tile([128, QTT * 8], I16)
            nc.sync.dma_start(q1_sb[:], qidx1[:])
            # s_dst per rank, produced by phase A, consumed by phase B
            sdst1_sb = cpool.tile([128, NRB], F32, tag="sdst1")
            sdst2_sb = cpool.tile([128, NRB], F32, tag="sdst2")

            # ------- phase A: h_aug = x @ [W | W@a_dst] for own nodes -------
            def phase_a(waug_sb, layer, sdst_sb):
                with tc.tile_pool(name="pa", bufs=3) as pa, \
                     tc.tile_pool(name="pap", bufs=2, space="PSUM") as pap:
                    j = 0
                    while j < NSH:
                        w = min(512, NSH - j)
                        xt = pa.tile([128, 512], F32, tag="pa_xt")
                        if layer == 1:
                            nc.sync.dma_start(xt[:, 0:w], embTk[:, j:j + w])
                        else:
                            nc.sync.dma_start(xt[:, 0:w], x1T_sh[:, j:j + w])
                        nt = (w + 127) // 128
                        for t in range(nt):
                            tw = min(128, w - t * 128)
                            ps = pap.tile([128, C + 1], F32, tag="pa_ps")
                            nc.tensor.matmul(ps[0:tw, :],
                                             xt[:, t * 128:t * 128 + tw],
                                             waug_sb[:], start=True,
                                             stop=True)
                            hs = pa.tile([128, C], F16, tag="pa_hs")
                            nc.scalar.activation(hs[0:tw, :], ps[0:tw, 0:C],
                                                 AF.Copy)
                            col = j // 128 + t
                            nc.vector.tensor_copy(sdst_sb[0:tw, col:col + 1],
                                                  ps[0:tw, C:C + 1])
                            nc.sync.dma_start(
                                h_sh[j + t * 128:j + t * 128 + tw, :],
                                hs[0:tw, :])
                        j += w

            # ---------------- phase B: edge processing ----------------
            def phase_b(layer, a_sb, sdst_sb, h_full):
                with tc.tile_pool(name="pb", bufs=TUNE["pb_bufs"]) as pb, \
                     tc.tile_pool(name="pbs", bufs=2) as pbs, \
                     tc.tile_pool(name="pbp", bufs=2, space="PSUM") as pbp:
                    trash = pbs.tile([128, 128], F16, tag="trash")
                    for ch in chunks:
                        nt = ch.nA + ch.nB
                        nrb = len(ch.rbs)
                        G = pb.tile([128, NTmax, C], F16, tag="G")
                        if ch.nA:
                            nc.gpsimd.dma_gather(
                                G[:, 0:ch.nA, :], h_full[0:HALF, :],
                                gidx_sb[:, ch.gcolA:ch.gcolA + ch.nA * 8],
                                ch.nA * 128, ch.nA * 128, C,
                                single_packet=False)
                        if ch.nB:
                            nc.gpsimd.dma_gather(
                                G[:, ch.nA:nt, :], h_full[HALF:N, :],
                                gidx_sb[:, ch.gcolB:ch.gcolB + ch.nB * 8],
                                ch.nB * 128, ch.nB * 128, C,
                                single_packet=False, queue_num=1)
                        oh_sb = pb.tile([128, NTmax * 128], F8, tag="oh")
                        nc.sync.dma_start(
                            oh_sb[:, 0:nt * 128],
                            ohA[:, ch.tilebase * 128:(ch.tilebase + nt) * 128])
                        ohT_sb = pb.tile([128, NTmax * 128], F8, tag="ohT")
                        nc.sync.dma_start(
                            ohT_sb[:, 0:nt * 128],
                            ohTA[:, ch.tilebase * 128:(ch.tilebase + nt) * 128])
                        # s_dst per rank block: straight from phase A
                        sdb = pbs.tile([128, RBS_PER_CHUNK], BF16, tag="sdb")
                        r0 = ch.rbs[0]
                        nc.vector.tensor_copy(sdb[:, 0:nrb],
                                              sdst_sb[:, r0:r0 + nrb])
                        # per-tile: s_src dot + s_dst broadcast matmul
                        ssrc = pbs.tile([128, NTmax], F32, tag="ssrc")
                        ps_sd = pbp.tile([128, NTmax], F32, tag="ps_sd")
                        for t, tl in enumerate(ch.tiles):
                            nc.vector.scalar_tensor_tensor(
                                trash[:], G[:, t, :], 1.0, a_sb[:],
                                OP.mult, OP.mult, accum_out=ssrc[:, t:t + 1])
                            nc.tensor.matmul(
                                ps_sd[:, t:t + 1],
                                ohT_sb[:, t * 128:(t + 1) * 128],
                                sdb[:, tl.rbslot:tl.rbslot + 1],
                                start=True, stop=True, skip_group_check=True)
                        z = pbs.tile([128, NTmax], F32, tag="z")
                        nc.vector.tensor_add(z[:, 0:nt], ssrc[:, 0:nt],
                                             ps_sd[:, 0:nt])
                        lr = pbs.tile([128, NTmax], F32, tag="lr")
                        # leaky relu on DVE: max(0.2*z, z)
                        nc.vector.scalar_tensor_tensor(
                            lr[:, 0:nt], z[:, 0:nt], 0.2, z[:, 0:nt],
                            OP.mult, OP.max)
                        w_sb = pbs.tile([128, NTmax], F32, tag="w")
                        nc.scalar.activation(w_sb[:, 0:nt], lr[:, 0:nt],
                                             AF.Exp)
                        rhs = pb.tile([128, NTmax, C + 1], BF16, tag="rhs")
                        for t in range(nt):
                            # per-edge scale, split between DVE and Act
                            if t % TUNE["dve_mod"] < TUNE["dve_thr"]:
                                nc.vector.tensor_scalar(
                                    rhs[:, t, 0:C], G[:, t, :],
                                    w_sb[:, t:t + 1], None, OP.mult)
                            else:
                                nc.scalar.activation(rhs[:, t, 0:C],
                                                     G[:, t, :], AF.Copy,
                                                     scale=w_sb[:, t:t + 1])
                        nc.vector.tensor_copy(rhs[:, 0:nt, C], w_sb[:, 0:nt])
                        ps_pack = pbp.tile([128, RBS_PER_CHUNK, C + 1], F32,
                                           tag="ps_pack")
                        for t, tl in enumerate(ch.tiles):
                            nc.tensor.matmul(
                                ps_pack[:, tl.rbslot, :],
                                oh_sb[:, t * 128:(t + 1) * 128],
                                rhs[:, t, :],
                                start=tl.first, stop=tl.last,
                                skip_group_check=True)
                        # epilogue per rank block
                        for i, r in enumerate(ch.rbs):
                            cap = min(RB, NSH - r * RB)
                            dn = pbs.tile([128, 1], F32, tag="dn")
                            nc.vector.tensor_scalar_add(
                                dn[:], ps_pack[:, i, C:C + 1], 1e-16)
                            rcp = pbs.tile([128, 1], F32, tag="rcp")
                            nc.vector.reciprocal(rcp[:], dn[:])
                            if layer == 1:
                                x1b = pbs.tile([128, C], F32, tag="x1b")
                                if fast1:
                                    nc.scalar.activation(
                                        x1b[:], ps_pack[:, i, 0:C], AF.Relu,
                                        scale=rcp[:])
                                else:
                                    y = pbs.tile([128, C], F32, tag="y")
                                    nc.vector.scalar_tensor_tensor(
                                        y[:], ps_pack[:, i, 0:C], rcp[:],
                                        A_sb[:], OP.mult, OP.mult)
                                    y2 = pbs.tile([128, C], F32, tag="y2")
                                    nc.vector.tensor_add(y2[:], y[:],
                                                         C1_sb[:])
                                    nc.scalar.activation(x1b[:], y2[:],
                                                         AF.Relu)
                                ps_t = pbp.tile([128, 128], F32, tag="ps_t")
                                nc.tensor.transpose(ps_t[:], x1b[:], id_sb[:])
                                x1t = pbs.tile([128, 128], F32, tag="x1t")
                                nc.scalar.activation(x1t[:], ps_t[:], AF.Copy)
                                nc.sync.dma_start(
                                    x1T_sh[:, r * RB:r * RB + cap],
                                    x1t[:, 0:cap])
                            else:
                                x2b = pbs.tile([128, C], F16, tag="x2b")
                                nc.vector.scalar_tensor_tensor(
                                    x2b[:], ps_pack[:, i, 0:C], rcp[:],
                                    b2_sb[:], OP.mult, OP.add)
                                nc.sync.dma_start(
                                    x2_sh[r * RB:r * RB + cap, :],
                                    x2b[0:cap, :])

            # ---------------- run the stages ----------------
            def allgather(src, dst):
                nc.gpsimd.collective_compute(
                    "AllGather", OP.bypass,
                    replica_groups=[list(range(NCORES))],
                    ins=[src.ap().opt()], outs=[dst.ap().opt()])

            stages = ["A1", "AGh1", "B1", "A2", "AGh2", "B2", "AGx2"]
            cut = stages.index(upto) if upto in stages else len(stages)

            if cut >= 0:
                with nc.named_scope("A1"):
                    phase_a(W1_sb, 1, sdst1_sb)
            if cut >= 1:
                with nc.named_scope("AGh1"):
                    allgather(h_sh, h_full1)
            if cut >= 2:
                with nc.named_scope("B1"):
                    phase_b(1, a1_sb, sdst1_sb, h_full1)
            if cut >= 3:
                with nc.named_scope("A2"):
                    phase_a(W2_sb, 2, sdst2_sb)
            if cut >= 4:
                with nc.named_scope("AGh2"):
                    allgather(h_sh, h_full2)
            if cut >= 5:
                with nc.named_scope("B2"):
                    phase_b(2, a2_sb, sdst2_sb, h_full2)
            if cut >= 6:
                with nc.named_scope("AGx2"):
                    allgather(x2_sh, x2_full)

            # ------- phase C: link predictor (transposed, batched) -------
            # gather x2 endpoint features transposed [c, q]; per 512-query
            # block: hq = U*V (DVE), z1 = Wp1^T @ hq + bp1 (PE, bias via a
            # K=1 matmul), relu (Act), out = wp2^T @ z1 (PE), sigmoid (Act).
            qgrp_tiles = S["qgrp_tiles"]
            NQmax = int(max(qgrp_tiles)) * 128
            with nc.named_scope("C"), \
                 tc.tile_pool(name="pc", bufs=2) as pc, \
                 tc.tile_pool(name="pcs", bufs=2) as pcs, \
                 tc.tile_pool(name="pcp", bufs=2, space="PSUM") as pcp:
                t0 = 0
                for gi in range(4 if cut >= 7 else 0):
                    ng = int(qgrp_tiles[gi]) * 128
                    if ng == 0:
                        continue
                    b0 = (gi >> 1) * HALF
                    b1 = (gi & 1) * HALF
                    U = pc.tile([128, 1, NQmax], F16, tag="U")
                    V = pc.tile([128, 1, NQmax], F16, tag="V")
                    nc.gpsimd.dma_gather(
                        U[:, :, 0:ng], x2_full[b0:b0 + HALF, :],
                        q0_sb[:, t0 * 8:t0 * 8 + ng // 16],
                        ng, ng, C, transpose=True, single_packet=False)
                    nc.gpsimd.dma_gather(
                        V[:, :, 0:ng], x2_full[b1:b1 + HALF, :],
                        q1_sb[:, t0 * 8:t0 * 8 + ng // 16],
                        ng, ng, C, transpose=True, single_packet=False,
                        queue_num=1)
                    res = pcs.tile([1, NQmax], F32, tag="res")
                    for o in range(0, ng, 512):
                        blk = min(512, ng - o)
                        hq = pcs.tile([128, 512], BF16, tag="hq")
                        nc.vector.tensor_mul(hq[:, 0:blk], U[:, 0, o:o + blk],
                                             V[:, 0, o:o + blk])
                        ps_z = pcp.tile([128, 512], F32, tag="ps_z")
                        nc.tensor.matmul(ps_z[:, 0:blk], bp1_sb[:],
                                         ones_sb[:, 0:blk],
                                         start=True, stop=False,
                                         skip_group_check=True)
                        nc.tensor.matmul(ps_z[:, 0:blk], Wp1_sb[:],
                                         hq[:, 0:blk],
                                         start=False, stop=True,
                                         skip_group_check=True)
                        zr = pcs.tile([128, 512], BF16, tag="zr")
                        nc.scalar.activation(zr[:, 0:blk], ps_z[:, 0:blk],
                                             AF.Relu)
                        ps_o = pcp.tile([128, 512], F32, tag="ps_o")
                        nc.tensor.matmul(ps_o[0:1, 0:blk], wp2_sb[:],
                                         zr[:, 0:blk],
                                         start=True, stop=True,
                                         skip_group_check=True)
                        nc.scalar.activation(res[:, o:o + blk],
                                             ps_o[0:1, 0:blk],
                                             AF.Sigmoid, bias=float(bp2val))
                    nc.sync.dma_start(out_q[:, t0 * 128:t0 * 128 + ng],
                                      res[:, 0:ng])
                    t0 += int(qgrp_tiles[gi])

    nc.compile()
    return nc


# ------------------------------------------------------------------- kernel

_CACHE = {}
LAST_RESULT = None


def build_all(inputs):
    """Host prep + program build + per-core input maps. Returns
    (nc, in_maps, S) for kernel() and for external bench harnesses."""
    inputs = {k: np.asarray(v) for k, v in inputs.items()}
    S = _prep(inputs)

    gamma = inputs["gamma"].astype(np.float32)
    rvar = inputs["rvar"].astype(np.float32)
    rmean = inputs["rmean"].astype(np.float32)
    beta = inputs["beta"].astype(np.float32)
    b1 = inputs["b1"].astype(np.float32)
    A = gamma / np.sqrt(rvar + BN_EPS)
    C1 = (b1 - rmean) * A + beta
    fast1 = bool(np.allclose(A, 1.0) and np.allclose(C1, 0.0))
    bp2val = float(np.asarray(inputs["bp2"]).reshape(-1)[0])

    nc = _build_program(S, fast1, bp2val)

    perm = S["perm"]
    emb = inputs["embedding"].astype(np.float32)
    embT_p = np.ascontiguousarray(emb[perm].T)

    W1 = inputs["W1"].astype(np.float32)
    W2 = inputs["W2"].astype(np.float32)
    ad1 = inputs["a_dst1"].astype(np.float32)
    ad2 = inputs["a_dst2"].astype(np.float32)
    W1aug = np.ascontiguousarray(
        np.concatenate([W1, (W1 @ ad1)[:, None]], axis=1))
    W2aug = np.ascontiguousarray(
        np.concatenate([W2, (W2 @ ad2)[:, None]], axis=1))

    common = dict(
        W1aug=W1aug, W2aug=W2aug,
        a1rep=_rep16(inputs["a_src1"]),
        a2rep=_rep16(inputs["a_src2"]),
        Arep=_rep(A), C1rep=_rep(C1),
        b2rep=_rep(inputs["b2"]),
        Wp1bf=inputs["Wp1"].astype(np_bf16),
        wp2bf=np.ascontiguousarray(inputs["Wp2"].astype(np_bf16)),
        bp1bf=np.ascontiguousarray(
            inputs["bp1"].astype(np_bf16)[None, :]),
        ident=np.eye(128, dtype=np.float32),
    )

    in_maps = []
    for k in range(NCORES):
        ce, cq = S["core_edge"][k], S["core_q"][k]
        m = dict(common)
        m.update(embTk=np.ascontiguousarray(
                     embT_p[:, k * NSH:(k + 1) * NSH]),
                 ohA=ce["oh"], ohTA=ce["ohT"], gidx=ce["gidx"],
                 qidx0=cq["qidx0"], qidx1=cq["qidx1"])
        in_maps.append(m)

    return nc, in_maps, S


def unpack_output(results, S):
    out = np.zeros(Q, np.float32)
    for k in range(NCORES):
        vals = np.asarray(results[k]["out_q"])      # [1, QTT*128]
        flat = vals.reshape(-1)                     # gather order == qmap idx
        qmap = S["core_q"][k]["qmap"]
        valid = qmap >= 0
        out[qmap[valid]] = flat[valid]
    return out


def kernel(**inputs):
    global LAST_RESULT
    nc, in_maps, S = build_all(inputs)
    res = run_bass_kernel_spmd(nc, in_maps, list(range(NCORES)))
    LAST_RESULT = res
    return unpack_output(res.results, S)



# revision 10
# speedup vs baseline: 1.4042x; 1.4042x over previous
"""Trainium2 Bass kernel for a 2-layer GAT + link predictor (nn_GAT).

Strategy (8 NeuronCores, SPMD single program):
  - Nodes are permuted and assigned to (core, rank) slots so every 128-rank
    block carries a near-equal number of incoming edges (load balance AND
    uniform program structure across cores).
  - Per layer:
      phase A (dense):  h = x @ W via PE; h rows written to DRAM.
      phase B (edges, sharded by dst): dma_gather h[src] rows; per-edge
        attention logits via on-chip dots + fp8 one-hot matmuls; softmax
        without max-subtraction (mathematically identical); weighted
        aggregation + denominator accumulate in PSUM via fp8 one-hot
        matmuls.  Per-core rank rows of s_dst are fetched with an indexed
        gather (per-core index inputs keep the program core-independent).
  - AllGather (DRAM collective) of layer outputs between layers.
  - Link predictor sharded over queries, 4-way grouped by (e0,e1) int16
    halves.
All index-derived data (permutations, one-hot tiles, gather indices) is
precomputed on the host from edge_index/edges; weights-derived scalars are
folded on the host (BN fold, W@a_dst).
"""
import heapq

import numpy as np
import ml_dtypes

from concourse import bass, bacc, tile
from concourse.bass_utils import run_bass_kernel_spmd

mybir = bass.mybir
AF = mybir.ActivationFunctionType
OP = mybir.AluOpType

F32 = mybir.dt.float32
BF16 = mybir.dt.bfloat16
F16 = mybir.dt.float16
F8 = mybir.dt.float8e4
I16 = mybir.dt.int16

N, C, E, Q = 50000, 128, 800000, 200000
NCORES = 8
NSH = N // NCORES            # 6250 nodes/core
RB = 128                     # ranks per block
NRB = (NSH + RB - 1) // RB   # 49 blocks/core
HALF = N // 2
RBS_PER_CHUNK = 3
BN_EPS = 1e-5

np_bf16 = ml_dtypes.bfloat16
np_f8 = ml_dtypes.float8_e4m3


# ----------------------------------------------------------------- host prep

def _build_perm(dst):
    """node -> (core, rank) assignment balancing edges per rank-block."""
    deg = np.bincount(dst, minlength=N)
    order = np.argsort(-deg, kind="stable")
    nblocks = NCORES * NRB
    caps = np.array([min(RB, NSH - rb * RB)
                     for k in range(NCORES) for rb in range(NRB)], np.int64)
    cnt = np.zeros(nblocks, np.int64)
    fill = [[] for _ in range(nblocks)]
    heap = [(0, b) for b in range(nblocks)]
    heapq.heapify(heap)
    for n in order:
        while True:
            load, b = heapq.heappop(heap)
            if cnt[b] < caps[b]:
                break
        fill[b].append(n)
        cnt[b] += 1
        if cnt[b] < caps[b]:
            heapq.heappush(heap, (load + int(deg[n]), b))
    perm = np.concatenate([np.array(fill[b], np.int64) for b in range(nblocks)])
    iperm = np.zeros(N, np.int64)
    iperm[perm] = np.arange(N)
    return perm, iperm


def _wrap_idx(idx):
    """[n] int -> [128, n//16] int16 (wrapped in 16 partitions, replicated)."""
    a = np.asarray(idx, np.int16).reshape(-1, 16).T
    return np.ascontiguousarray(np.tile(a, (8, 1)))


class _Tile:
    __slots__ = ("rb", "rbslot", "first", "last", "gslot")

    def __init__(self, rb, rbslot, first, last, gslot):
        self.rb, self.rbslot = rb, rbslot
        self.first, self.last = first, last
        self.gslot = gslot


class _Chunk:
    __slots__ = ("rbs", "tiles", "nA", "nB", "gcolA", "gcolB", "tilebase",
                 "rbcol")

    def __init__(self):
        self.rbs, self.tiles = [], []


def _prep(inputs):
    """All host-side index preprocessing. Returns a structure dict."""
    src = np.asarray(inputs["edge_index"][0], np.int64)
    dst = np.asarray(inputs["edge_index"][1], np.int64)
    perm, iperm = _build_perm(dst)
    src_p, dst_p = iperm[src], iperm[dst]

    dst_core = dst_p // NSH
    percore = []
    ntile_rbh = np.zeros((NRB, 2), np.int64)
    for k in range(NCORES):
        m = dst_core == k
        s, d = src_p[m], dst_p[m]
        rank = d - k * NSH
        rb = rank // RB
        half = (s >= HALF).astype(np.int64)
        percore.append((s, rank, rb, half))
        for r in range(NRB):
            c0 = int(((rb == r) & (half == 0)).sum())
            c1 = int(((rb == r) & (half == 1)).sum())
            ntile_rbh[r, 0] = max(ntile_rbh[r, 0], (c0 + 127) // 128)
            ntile_rbh[r, 1] = max(ntile_rbh[r, 1], (c1 + 127) // 128)
    ntile_rbh = np.maximum(ntile_rbh, 1)
    T = int(ntile_rbh.sum())

    # chunk structure (same for all cores)
    chunks = []
    tilebase = 0
    for c0 in range(0, NRB, RBS_PER_CHUNK):
        ch = _Chunk()
        ch.rbs = list(range(c0, min(c0 + RBS_PER_CHUNK, NRB)))
        ch.tilebase = tilebase
        gslot = 0
        for h in (0, 1):
            for rbslot, r in enumerate(ch.rbs):
                nt = int(ntile_rbh[r, h])
                for j in range(nt):
                    first = (h == 0 and j == 0)
                    last = (h == 1 and j == nt - 1)
                    ch.tiles.append(_Tile(r, rbslot, first, last, gslot))
                    gslot += 1
            if h == 0:
                ch.nA = gslot
        ch.nB = gslot - ch.nA
        tilebase += gslot
        chunks.append(ch)
    assert tilebase == T
    NTmax = max(ch.nA + ch.nB for ch in chunks)

    # gidx column ranges per (chunk, half): laid consecutively
    col = 0
    for ch in chunks:
        ch.gcolA = col
        col += ch.nA * 8
        ch.gcolB = col
        col += ch.nB * 8
    gcols = col
    # rb gather idx columns: 8 per rb, chunk slice = rbs
    for ch in chunks:
        ch.rbcol = ch.rbs[0] * 8

    # per-core edge data
    core_edge = []
    for k in range(NCORES):
        s, rank, rb, half = percore[k]
        gidx = np.zeros((128, gcols), np.int16)
        oh = np.zeros((128, T * 128), np_f8)
        ohT = np.zeros((128, T * 128), np_f8)
        for ch in chunks:
            for h in (0, 1):
                colbase = ch.gcolA if h == 0 else ch.gcolB
                slot0 = 0 if h == 0 else ch.nA
                nslots = ch.nA if h == 0 else ch.nB
                idxs = np.zeros(nslots * 128, np.int64)
                pos = 0
                for r in ch.rbs:
                    m2 = (rb == r) & (half == h)
                    ss = s[m2] - h * HALF
                    rk = rank[m2] - r * RB
                    nt = int(ntile_rbh[r, h])
                    idxs[pos:pos + len(ss)] = ss
                    # one-hot data, global tile index
                    gt0 = ch.tilebase + slot0 + pos // 128
                    e_in = np.arange(len(ss))
                    gpos = pos + e_in           # slot-local position
                    tt = ch.tilebase + slot0 + gpos // 128
                    pp = gpos % 128
                    oh[pp, tt * 128 + rk] = 1.0
                    ohT[rk, tt * 128 + pp] = 1.0
                    pos += nt * 128
                gidx[:, colbase:colbase + nslots * 8] = _wrap_idx(idxs)
        # rb row gathers: rank rows of this core, half-local with dummy 0
        base_rows = k * NSH + np.arange(NRB * RB)
        rbr = np.minimum(base_rows, (k + 1) * NSH - 1)  # clamp pad rows
        if k < 4:
            ra, rbb = rbr, np.zeros_like(rbr)
        else:
            ra, rbb = np.zeros_like(rbr), rbr - HALF
        core_edge.append(dict(
            gidx=gidx, oh=oh, ohT=ohT,
            rbidxA=_wrap_idx(ra), rbidxB=_wrap_idx(rbb),
        ))

    # ---- queries
    e0 = iperm[np.asarray(inputs["edges"][0], np.int64)]
    e1 = iperm[np.asarray(inputs["edges"][1], np.int64)]
    QSH = Q // NCORES
    qgrp_tiles = np.zeros(4, np.int64)
    qcore = []
    for k in range(NCORES):
        sl = slice(k * QSH, (k + 1) * QSH)
        a, b = e0[sl], e1[sl]
        g = (a >= HALF).astype(np.int64) * 2 + (b >= HALF).astype(np.int64)
        qcore.append((a, b, g))
        for gi in range(4):
            cnt = int((g == gi).sum())
            qgrp_tiles[gi] = max(qgrp_tiles[gi], (cnt + 127) // 128)
    QTT = int(qgrp_tiles.sum())
    # subchunks of <=32 tiles per group
    qchunks = []   # (group, tile0_in_out, ntiles)
    tpos = 0
    for gi in range(4):
        nt = int(qgrp_tiles[gi])
        j = 0
        while j < nt:
            step = min(32, nt - j)
            qchunks.append((gi, tpos + j, step))
            j += step
        tpos += nt

    core_q = []
    for k in range(NCORES):
        a, b, g = qcore[k]
        qi0 = np.zeros(QTT * 128, np.int64)
        qi1 = np.zeros(QTT * 128, np.int64)
        qmap = np.full(QTT * 128, -1, np.int64)
        tpos = 0
        for gi in range(4):
            m = g == gi
            cnt = int(m.sum())
            qi0[tpos:tpos + cnt] = a[m] - (gi >> 1) * HALF
            qi1[tpos:tpos + cnt] = b[m] - (gi & 1) * HALF
            qmap[tpos:tpos + cnt] = np.nonzero(m)[0] + k * QSH
            tpos += int(qgrp_tiles[gi]) * 128
        core_q.append(dict(qidx0=_wrap_idx(qi0), qidx1=_wrap_idx(qi1),
                           qmap=qmap))

    return dict(perm=perm, iperm=iperm, chunks=chunks, ntile_rbh=ntile_rbh,
                T=T, NTmax=NTmax, gcols=gcols, core_edge=core_edge,
                qgrp_tiles=qgrp_tiles, qchunks=qchunks, QTT=QTT,
                core_q=core_q)


def _rep(v):
    """[C] -> [128, C] replicated fp32."""
    return np.ascontiguousarray(np.broadcast_to(
        np.asarray(v, np.float32)[None, :], (128, C)))


def _rep16(v):
    """[C] -> [128, C] replicated fp16."""
    return np.ascontiguousarray(np.broadcast_to(
        np.asarray(v, np.float16)[None, :], (128, C)))


def _repbf(v):
    """[C] -> [128, C] replicated bf16."""
    return np.ascontiguousarray(np.broadcast_to(
        np.asarray(v, np.float32).astype(np_bf16)[None, :], (128, C)))


# ------------------------------------------------------------ program build

TUNE = dict(pb_bufs=3, dve_mod=1, dve_thr=0)


def _build_program(S, fast1, bp2val, upto=None):
    """Build the SPMD Bass program. S is the _prep structure.
    upto: truncate after the named stage (HW bisection; output stays zero).

    Sharded design: each core computes h (and s_dst) only for its own NSH
    nodes; fp16 h shards are AllGathered into a Shared h_full that the
    edge-phase gathers read. s_dst per rank block comes straight out of the
    phase-A matmul (W_aug = [W | W@a_dst]) and stays in SBUF."""
    nc = bacc.Bacc("TRN2", target_bir_lowering=False, debug=False,
                   num_devices=NCORES, num_swdge_queues=2)
    T, NTmax, QTT = S["T"], S["NTmax"], S["QTT"]
    chunks, qchunks = S["chunks"], S["qchunks"]

    def din(name, shape, dt):
        return nc.dram_tensor(name, shape, dt, kind="ExternalInput")

    # inputs
    embTk = din("embTk", [128, NSH], BF16)       # per-core node slice
    W1aug = din("W1aug", [128, C + 1], BF16)     # [W1 | W1@a_dst1]
    W2aug = din("W2aug", [128, C + 1], BF16)
    a1rep = din("a1rep", [128, C], BF16)
    a2rep = din("a2rep", [128, C], BF16)
    Arep = din("Arep", [128, C], F32)
    C1rep = din("C1rep", [128, C], F32)
    b2rep = din("b2rep", [128, C], F32)
    Wp1bf = din("Wp1bf", [128, C], BF16)
    wp2bf = din("wp2bf", [128, 1], BF16)
    bp1col = din("bp1col", [128, 1], F32)
    ident = din("ident", [128, 128], F32)
    ohA = din("ohA", [128, T * 128], F8)
    ohTA = din("ohTA", [128, T * 128], F8)
    gidx = din("gidx", [128, S["gcols"]], I16)
    qidx0 = din("qidx0", [128, QTT * 8], I16)
    qidx1 = din("qidx1", [128, QTT * 8], I16)

    out_q = nc.dram_tensor("out_q", [1, QTT * 128], F32,
                           kind="ExternalOutput")

    # internal DRAM (h tables bf16: halves gather traffic + 2x DVE rate)
    h_sh = nc.dram_tensor("h_sh", [NSH, C], BF16)
    h_full1 = nc.dram_tensor("h_full1", [N, C], BF16, addr_space="Shared")
    h_full2 = nc.dram_tensor("h_full2", [N, C], BF16, addr_space="Shared")
    x1T_sh = nc.dram_tensor("x1T_sh", [128, NSH], BF16)
    x2_sh = nc.dram_tensor("x2_sh", [NSH, C], BF16)
    x2_full = nc.dram_tensor("x2_full", [N, C], BF16, addr_space="Shared")

    with tile.TileContext(nc) as tc:
        with tc.tile_pool(name="const", bufs=1) as cpool:
            def load_const(t, w=C, dt=F32):
                sb = cpool.tile([128, w], dt, tag=t.name)
                nc.sync.dma_start(sb[:], t[:])
                return sb

            W1_sb = load_const(W1aug, w=C + 1, dt=BF16)
            W2_sb = load_const(W2aug, w=C + 1, dt=BF16)
            a1_sb = load_const(a1rep, dt=BF16)
            a2_sb = load_const(a2rep, dt=BF16)
            b2_sb = load_const(b2rep)
            Wp1_sb = load_const(Wp1bf, dt=BF16)
            wp2_sb = load_const(wp2bf, w=1, dt=BF16)
            id_sb = load_const(ident, 128)
            bp1_sb = cpool.tile([128, 1], F32, tag="bp1col")
            nc.sync.dma_start(bp1_sb[:], bp1col[:])
            onecol_sb = cpool.tile([128, 1], BF16, tag="onecol")
            nc.gpsimd.memset(onecol_sb[:], 1.0)
            if not fast1:
                A_sb = load_const(Arep)
                C1_sb = load_const(C1rep)
            gidx_sb = cpool.tile([128, S["gcols"]], I16)
            nc.sync.dma_start(gidx_sb[:], gidx[:])
            q0_sb = cpool.tile([128, QTT * 8], I16)
            nc.sync.dma_start(q0_sb[:], qidx0[:])
            q1_sb = cpool.tile([128, QTT * 8], I16)
            nc.sync.dma_start(q1_sb[:], qidx1[:])
            # s_dst per rank, produced by phase A, consumed by phase B.
            # memset first: phase A leaves pad ranks (NSH % 128 tail) of the
            # last rank block unwritten, and garbage there turns into NaN via
            # 0 * NaN in the s_dst broadcast matmul.
            sdst1_sb = cpool.tile([128, NRB], F32, tag="sdst1")
            nc.gpsimd.memset(sdst1_sb[:], 0.0)
            sdst2_sb = cpool.tile([128, NRB], F32, tag="sdst2")
            nc.gpsimd.memset(sdst2_sb[:], 0.0)

            # ------- phase A: h_aug = x @ [W | W@a_dst] for own nodes -------
            def phase_a(waug_sb, layer, sdst_sb):
                with tc.tile_pool(name="pa", bufs=3) as pa, \
                     tc.tile_pool(name="pap", bufs=2, space="PSUM") as pap:
                    j = 0
                    while j < NSH:
                        w = min(512, NSH - j)
                        xt = pa.tile([128, 512], BF16, tag="pa_xt")
                        if layer == 1:
                            nc.sync.dma_start(xt[:, 0:w], embTk[:, j:j + w])
                        else:
                            nc.sync.dma_start(xt[:, 0:w], x1T_sh[:, j:j + w])
                        nt = (w + 127) // 128
                        for t in range(nt):
                            tw = min(128, w - t * 128)
                            ps = pap.tile([128, C + 1], F32, tag="pa_ps")
                            nc.tensor.matmul(ps[0:tw, :],
                                             xt[:, t * 128:t * 128 + tw],
                                             waug_sb[:], start=True,
                                             stop=True)
                            hs = pa.tile([128, C], BF16, tag="pa_hs")
                            nc.scalar.activation(hs[0:tw, :], ps[0:tw, 0:C],
                                                 AF.Copy)
                            col = j // 128 + t
                            nc.vector.tensor_copy(sdst_sb[0:tw, col:col + 1],
                                                  ps[0:tw, C:C + 1])
                            nc.sync.dma_start(
                                h_sh[j + t * 128:j + t * 128 + tw, :],
                                hs[0:tw, :])
                        j += w

            # ---------------- phase B: edge processing ----------------
            def phase_b(layer, a_sb, sdst_sb, h_full):
                with tc.tile_pool(name="pb", bufs=TUNE["pb_bufs"]) as pb, \
                     tc.tile_pool(name="pbs", bufs=2) as pbs, \
                     tc.tile_pool(name="pot", bufs=8) as pot, \
                     tc.tile_pool(name="pbp", bufs=2, space="PSUM") as pbp:
                    for ch in chunks:
                        nt = ch.nA + ch.nB
                        nrb = len(ch.rbs)
                        G = pb.tile([128, NTmax, C], BF16, tag="G")
                        if ch.nA:
                            nc.gpsimd.dma_gather(
                                G[:, 0:ch.nA, :], h_full[0:HALF, :],
                                gidx_sb[:, ch.gcolA:ch.gcolA + ch.nA * 8],
                                ch.nA * 128, ch.nA * 128, C,
                                single_packet=False)
                        if ch.nB:
                            nc.gpsimd.dma_gather(
                                G[:, ch.nA:nt, :], h_full[HALF:N, :],
                                gidx_sb[:, ch.gcolB:ch.gcolB + ch.nB * 8],
                                ch.nB * 128, ch.nB * 128, C,
                                single_packet=False, queue_num=1)
                        oh_sb = pb.tile([128, NTmax * 128], F8, tag="oh")
                        nc.sync.dma_start(
                            oh_sb[:, 0:nt * 128],
                            ohA[:, ch.tilebase * 128:(ch.tilebase + nt) * 128])
                        ohT_sb = pb.tile([128, NTmax * 128], F8, tag="ohT")
                        nc.sync.dma_start(
                            ohT_sb[:, 0:nt * 128],
                            ohTA[:, ch.tilebase * 128:(ch.tilebase + nt) * 128])
                        # s_dst per rank block: straight from phase A
                        sdb = pbs.tile([128, RBS_PER_CHUNK], BF16, tag="sdb")
                        r0 = ch.rbs[0]
                        nc.vector.tensor_copy(sdb[:, 0:nrb],
                                              sdst_sb[:, r0:r0 + nrb])
                        # batched s_src: prod = G * a (bcast), reduce inner C
                        prod = pb.tile([128, NTmax, C], BF16, tag="prod")
                        a_bc = a_sb[:].unsqueeze(1).broadcast_to((128, nt, C))
                        nc.vector.tensor_mul(prod[:, 0:nt, :],
                                             G[:, 0:nt, :], a_bc)
                        ssrc = pbs.tile([128, NTmax], F32, tag="ssrc")
                        nc.vector.tensor_reduce(
                            ssrc[:, 0:nt], prod[:, 0:nt, :],
                            axis=mybir.AxisListType.X, op=OP.add)
                        # s_dst broadcast to edges (PE, N=1 per tile)
                        ps_sd = pbp.tile([128, NTmax], F32, tag="ps_sd")
                        for t, tl in enumerate(ch.tiles):
                            nc.tensor.matmul(
                                ps_sd[:, t:t + 1],
                                ohT_sb[:, t * 128:(t + 1) * 128],
                                sdb[:, tl.rbslot:tl.rbslot + 1],
                                start=True, stop=True, skip_group_check=True)
                        z = pbs.tile([128, NTmax], F32, tag="z")
                        nc.vector.tensor_add(z[:, 0:nt], ssrc[:, 0:nt],
                                             ps_sd[:, 0:nt])
                        lr = pbs.tile([128, NTmax], F32, tag="lr")
                        # leaky relu on DVE: max(0.2*z, z)
                        nc.vector.scalar_tensor_tensor(
                            lr[:, 0:nt], z[:, 0:nt], 0.2, z[:, 0:nt],
                            OP.mult, OP.max)
                        w_bf = pbs.tile([128, NTmax], F32, tag="w")
                        nc.scalar.activation(w_bf[:, 0:nt], lr[:, 0:nt],
                                             AF.Exp)
                        # aggregation: fold w into the one-hot stationary
                        ps_pack = pbp.tile([128, RBS_PER_CHUNK, C], F32,
                                           tag="ps_pack")
                        ps_den = pbp.tile([128, RBS_PER_CHUNK], F32,
                                          tag="ps_den")
                        for t, tl in enumerate(ch.tiles):
                            ohw = pot.tile([128, 128], BF16, tag="ohw")
                            nc.vector.tensor_scalar(
                                ohw[:], oh_sb[:, t * 128:(t + 1) * 128],
                                w_bf[:, t:t + 1], None, OP.mult)
                            nc.tensor.matmul(
                                ps_pack[:, tl.rbslot, :],
                                ohw[:], G[:, t, :],
                                start=tl.first, stop=tl.last,
                                skip_group_check=True)
                            nc.tensor.matmul(
                                ps_den[:, tl.rbslot:tl.rbslot + 1],
                                ohw[:], onecol_sb[:],
                                start=tl.first, stop=tl.last,
                                skip_group_check=True)
                        # epilogue per rank block
                        for i, r in enumerate(ch.rbs):
                            cap = min(RB, NSH - r * RB)
                            dn = pbs.tile([128, 1], F32, tag="dn")
                            nc.vector.tensor_scalar_add(
                                dn[:], ps_den[:, i:i + 1], 1e-16)
                            rcp = pbs.tile([128, 1], F32, tag="rcp")
                            nc.vector.reciprocal(rcp[:], dn[:])
                            if layer == 1:
                                x1b = pbs.tile([128, C], F32, tag="x1b")
                                if fast1:
                                    nc.scalar.activation(
                                        x1b[:], ps_pack[:, i, :], AF.Relu,
                                        scale=rcp[:])
                                else:
                                    y = pbs.tile([128, C], F32, tag="y")
                                    nc.vector.scalar_tensor_tensor(
                                        y[:], ps_pack[:, i, :], rcp[:],
                                        A_sb[:], OP.mult, OP.mult)
                                    y2 = pbs.tile([128, C], F32, tag="y2")
                                    nc.vector.tensor_add(y2[:], y[:],
                                                         C1_sb[:])
                                    nc.scalar.activation(x1b[:], y2[:],
                                                         AF.Relu)
                                ps_t = pbp.tile([128, 128], F32, tag="ps_t")
                                nc.tensor.transpose(ps_t[:], x1b[:], id_sb[:])
                                x1t = pbs.tile([128, 128], BF16, tag="x1t")
                                nc.scalar.activation(x1t[:], ps_t[:], AF.Copy)
                                nc.sync.dma_start(
                                    x1T_sh[:, r * RB:r * RB + cap],
                                    x1t[:, 0:cap])
                            else:
                                x2b = pbs.tile([128, C], BF16, tag="x2b")
                                nc.vector.scalar_tensor_tensor(
                                    x2b[:], ps_pack[:, i, :], rcp[:],
                                    b2_sb[:], OP.mult, OP.add)
                                nc.sync.dma_start(
                                    x2_sh[r * RB:r * RB + cap, :],
                                    x2b[0:cap, :])

            # ---------------- run the stages ----------------
            def allgather(src, dst):
                nc.gpsimd.collective_compute(
                    "AllGather", OP.bypass,
                    replica_groups=[list(range(NCORES))],
                    ins=[src.ap().opt()], outs=[dst.ap().opt()])

            stages = ["A1", "AGh1", "B1", "A2", "AGh2", "B2", "AGx2"]
            cut = stages.index(upto) if upto in stages else len(stages)

            if cut >= 0:
                with nc.named_scope("A1"):
                    phase_a(W1_sb, 1, sdst1_sb)
            if cut >= 1:
                with nc.named_scope("AGh1"):
                    allgather(h_sh, h_full1)
            if cut >= 2:
                with nc.named_scope("B1"):
                    phase_b(1, a1_sb, sdst1_sb, h_full1)
            if cut >= 3:
                with nc.named_scope("A2"):
                    phase_a(W2_sb, 2, sdst2_sb)
            if cut >= 4:
                with nc.named_scope("AGh2"):
                    allgather(h_sh, h_full2)
            if cut >= 5:
                with nc.named_scope("B2"):
                    phase_b(2, a2_sb, sdst2_sb, h_full2)
            if cut >= 6:
                with nc.named_scope("AGx2"):
                    allgather(x2_sh, x2_full)

            # ------- phase C: link predictor (transposed, batched) -------
            # gather x2 endpoint features transposed [c, q]; per 512-query
            # block: hq = U*V (DVE), z1 = Wp1^T @ hq + bp1 (PE, bias via a
            # K=1 matmul), relu (Act), out = wp2^T @ z1 (PE), sigmoid (Act).
            qgrp_tiles = S["qgrp_tiles"]
            NQmax = int(max(qgrp_tiles)) * 128
            with nc.named_scope("C"), \
                 tc.tile_pool(name="pc", bufs=2) as pc, \
                 tc.tile_pool(name="pcs", bufs=2) as pcs, \
                 tc.tile_pool(name="pcp", bufs=2, space="PSUM") as pcp:
                t0 = 0
                for gi in range(4 if cut >= 7 else 0):
                    ng = int(qgrp_tiles[gi]) * 128
                    if ng == 0:
                        continue
                    b0 = (gi >> 1) * HALF
                    b1 = (gi & 1) * HALF
                    U = pc.tile([128, 1, NQmax], BF16, tag="U")
                    V = pc.tile([128, 1, NQmax], BF16, tag="V")
                    nc.gpsimd.dma_gather(
                        U[:, :, 0:ng], x2_full[b0:b0 + HALF, :],
                        q0_sb[:, t0 * 8:t0 * 8 + ng // 16],
                        ng, ng, C, transpose=True, single_packet=False)
                    nc.gpsimd.dma_gather(
                        V[:, :, 0:ng], x2_full[b1:b1 + HALF, :],
                        q1_sb[:, t0 * 8:t0 * 8 + ng // 16],
                        ng, ng, C, transpose=True, single_packet=False,
                        queue_num=1)
                    res = pcs.tile([1, NQmax], F32, tag="res")
                    for o in range(0, ng, 512):
                        blk = min(512, ng - o)
                        hq = pcs.tile([128, 512], BF16, tag="hq")
                        nc.vector.tensor_mul(hq[:, 0:blk], U[:, 0, o:o + blk],
                                             V[:, 0, o:o + blk])
                        ps_z = pcp.tile([128, 512], F32, tag="ps_z")
                        nc.tensor.matmul(ps_z[:, 0:blk], Wp1_sb[:],
                                         hq[:, 0:blk],
                                         start=True, stop=True,
                                         skip_group_check=True)
                        zr = pcs.tile([128, 512], BF16, tag="zr")
                        nc.scalar.activation(zr[:, 0:blk], ps_z[:, 0:blk],
                                             AF.Relu, bias=bp1_sb[:])
                        ps_o = pcp.tile([128, 512], F32, tag="ps_o")
                        nc.tensor.matmul(ps_o[0:1, 0:blk], wp2_sb[:],
                                         zr[:, 0:blk],
                                         start=True, stop=True,
                                         skip_group_check=True)
                        nc.scalar.activation(res[:, o:o + blk],
                                             ps_o[0:1, 0:blk],
                                             AF.Sigmoid, bias=float(bp2val))
                    nc.sync.dma_start(out_q[:, t0 * 128:t0 * 128 + ng],
                                      res[:, 0:ng])
                    t0 += int(qgrp_tiles[gi])

    nc.compile()
    return nc


# ------------------------------------------------------------------- kernel

_CACHE = {}
LAST_RESULT = None


def build_all(inputs):
    """Host prep + program build + per-core input maps. Returns
    (nc, in_maps, S) for kernel() and for external bench harnesses."""
    inputs = {k: np.asarray(v) for k, v in inputs.items()}
    S = _prep(inputs)

    gamma = inputs["gamma"].astype(np.float32)
    rvar = inputs["rvar"].astype(np.float32)
    rmean = inputs["rmean"].astype(np.float32)
    beta = inputs["beta"].astype(np.float32)
    b1 = inputs["b1"].astype(np.float32)
    A = gamma / np.sqrt(rvar + BN_EPS)
    C1 = (b1 - rmean) * A + beta
    fast1 = bool(np.allclose(A, 1.0) and np.allclose(C1, 0.0))
    bp2val = float(np.asarray(inputs["bp2"]).reshape(-1)[0])

    nc = _build_program(S, fast1, bp2val)

    perm = S["perm"]
    emb = inputs["embedding"].astype(np.float32)
    embT_p = np.ascontiguousarray(emb[perm].T)

    W1 = inputs["W1"].astype(np.float32)
    W2 = inputs["W2"].astype(np.float32)
    ad1 = inputs["a_dst1"].astype(np.float32)
    ad2 = inputs["a_dst2"].astype(np.float32)
    W1aug = np.ascontiguousarray(
        np.concatenate([W1, (W1 @ ad1)[:, None]], axis=1)).astype(np_bf16)
    W2aug = np.ascontiguousarray(
        np.concatenate([W2, (W2 @ ad2)[:, None]], axis=1)).astype(np_bf16)

    common = dict(
        W1aug=W1aug, W2aug=W2aug,
        a1rep=_repbf(inputs["a_src1"]),
        a2rep=_repbf(inputs["a_src2"]),
        Arep=_rep(A), C1rep=_rep(C1),
        b2rep=_rep(inputs["b2"]),
        Wp1bf=inputs["Wp1"].astype(np_bf16),
        wp2bf=np.ascontiguousarray(inputs["Wp2"].astype(np_bf16)),
        bp1col=np.ascontiguousarray(
            inputs["bp1"].astype(np.float32)[:, None]),
        ident=np.eye(128, dtype=np.float32),
    )

    in_maps = []
    for k in range(NCORES):
        ce, cq = S["core_edge"][k], S["core_q"][k]
        m = dict(common)
        m.update(embTk=np.ascontiguousarray(
                     embT_p[:, k * NSH:(k + 1) * NSH]).astype(np_bf16),
                 ohA=ce["oh"], ohTA=ce["ohT"], gidx=ce["gidx"],
                 qidx0=cq["qidx0"], qidx1=cq["qidx1"])
        in_maps.append(m)

    return nc, in_maps, S


def unpack_output(results, S):
    out = np.zeros(Q, np.float32)
    for k in range(NCORES):
        vals = np.asarray(results[k]["out_q"])      # [1, QTT*128]
        flat = vals.reshape(-1)                     # gather order == qmap idx
        qmap = S["core_q"][k]["qmap"]
        valid = qmap >= 0
        out[qmap[valid]] = flat[valid]
    return out


def kernel(**inputs):
    global LAST_RESULT
    nc, in_maps, S = build_all(inputs)
    res = run_bass_kernel_spmd(nc, in_maps, list(range(NCORES)))
    LAST_RESULT = res
    return unpack_output(res.results, S)



# revision 17
# speedup vs baseline: 1.5535x; 1.1064x over previous
"""Trainium2 Bass kernel for a 2-layer GAT + link predictor (nn_GAT).

Strategy (8 NeuronCores, SPMD single program):
  - Nodes are permuted and assigned to (core, rank) slots so every 128-rank
    block carries a near-equal number of incoming edges (load balance AND
    uniform program structure across cores).
  - Per layer:
      phase A (dense): h_aug = x @ [W | W@a_src | W@a_dst] via PE; table
        rows [h | s_src | 1 | pad] (512B) written to DRAM. Random 256B
        gather reads are latency/IOPS-bound (~6 ns/row), so the extra
        256B rides free and removes all per-edge s_src compute and the
        separate denominator matmul.
      phase B (edges, sharded by dst): dma_gather per-edge table rows
        (4 SWDGE queues); s_dst broadcast rank->edge via fp8 one-hot
        N=1 matmuls; z/leaky-relu/exp batched per chunk; softmax without
        max-subtraction (valid: logits are O(0.5)); exp(z) folded into
        the one-hot stationary (DVE tensor_scalar) so the aggregation is
        one N=130 matmul per tile whose col C+1 is the denominator.
        Per-rank epilogues are software-pipelined one chunk behind so
        PSUM-read stalls don't head-block the DVE/Act queues.
  - AllGather (DRAM collective) of layer tables between layers.
  - Link predictor sharded over queries, 4-way grouped by (e0,e1) int16
    halves; bp1 applied as activation bias, sigmoid on PSUM rows.
All index-derived data (permutations, one-hot tiles, gather indices) is
precomputed on the host from edge_index/edges; weights-derived vectors are
folded on the host (BN fold, W@a_src, W@a_dst).
"""
import heapq

import numpy as np
import ml_dtypes

from concourse import bass, bacc, tile
from concourse.bass_utils import run_bass_kernel_spmd

mybir = bass.mybir
AF = mybir.ActivationFunctionType
OP = mybir.AluOpType

F32 = mybir.dt.float32
BF16 = mybir.dt.bfloat16
F16 = mybir.dt.float16
F8 = mybir.dt.float8e4
I16 = mybir.dt.int16

N, C, E, Q = 50000, 128, 800000, 200000
NCORES = 8
NSH = N // NCORES            # 6250 nodes/core
RB = 128                     # ranks per block
NRB = (NSH + RB - 1) // RB   # 49 blocks/core
HALF = N // 2
RBS_PER_CHUNK = 3
BN_EPS = 1e-5

np_bf16 = ml_dtypes.bfloat16
np_f8 = ml_dtypes.float8_e4m3


# ----------------------------------------------------------------- host prep

def _build_perm(dst):
    """node -> (core, rank) assignment balancing edges per rank-block."""
    deg = np.bincount(dst, minlength=N)
    order = np.argsort(-deg, kind="stable")
    nblocks = NCORES * NRB
    caps = np.array([min(RB, NSH - rb * RB)
                     for k in range(NCORES) for rb in range(NRB)], np.int64)
    cnt = np.zeros(nblocks, np.int64)
    fill = [[] for _ in range(nblocks)]
    heap = [(0, b) for b in range(nblocks)]
    heapq.heapify(heap)
    for n in order:
        while True:
            load, b = heapq.heappop(heap)
            if cnt[b] < caps[b]:
                break
        fill[b].append(n)
        cnt[b] += 1
        if cnt[b] < caps[b]:
            heapq.heappush(heap, (load + int(deg[n]), b))
    perm = np.concatenate([np.array(fill[b], np.int64) for b in range(nblocks)])
    iperm = np.zeros(N, np.int64)
    iperm[perm] = np.arange(N)
    return perm, iperm


def _wrap_idx(idx):
    """[n] int -> [128, n//16] int16 (wrapped in 16 partitions, replicated)."""
    a = np.asarray(idx, np.int16).reshape(-1, 16).T
    return np.ascontiguousarray(np.tile(a, (8, 1)))


class _Tile:
    __slots__ = ("rb", "rbslot", "first", "last", "gslot")

    def __init__(self, rb, rbslot, first, last, gslot):
        self.rb, self.rbslot = rb, rbslot
        self.first, self.last = first, last
        self.gslot = gslot


class _Chunk:
    __slots__ = ("rbs", "tiles", "nA", "nB", "gcolA", "gcolB", "tilebase",
                 "rbcol")

    def __init__(self):
        self.rbs, self.tiles = [], []


def _prep(inputs):
    """All host-side index preprocessing. Returns a structure dict."""
    src = np.asarray(inputs["edge_index"][0], np.int64)
    dst = np.asarray(inputs["edge_index"][1], np.int64)
    perm, iperm = _build_perm(dst)
    src_p, dst_p = iperm[src], iperm[dst]

    dst_core = dst_p // NSH
    percore = []
    ntile_rbh = np.zeros((NRB, 2), np.int64)
    for k in range(NCORES):
        m = dst_core == k
        s, d = src_p[m], dst_p[m]
        rank = d - k * NSH
        rb = rank // RB
        half = (s >= HALF).astype(np.int64)
        percore.append((s, rank, rb, half))
        for r in range(NRB):
            c0 = int(((rb == r) & (half == 0)).sum())
            c1 = int(((rb == r) & (half == 1)).sum())
            ntile_rbh[r, 0] = max(ntile_rbh[r, 0], (c0 + 127) // 128)
            ntile_rbh[r, 1] = max(ntile_rbh[r, 1], (c1 + 127) // 128)
    ntile_rbh = np.maximum(ntile_rbh, 1)
    T = int(ntile_rbh.sum())

    # chunk structure (same for all cores)
    chunks = []
    tilebase = 0
    for c0 in range(0, NRB, RBS_PER_CHUNK):
        ch = _Chunk()
        ch.rbs = list(range(c0, min(c0 + RBS_PER_CHUNK, NRB)))
        ch.tilebase = tilebase
        gslot = 0
        for h in (0, 1):
            for rbslot, r in enumerate(ch.rbs):
                nt = int(ntile_rbh[r, h])
                for j in range(nt):
                    first = (h == 0 and j == 0)
                    last = (h == 1 and j == nt - 1)
                    ch.tiles.append(_Tile(r, rbslot, first, last, gslot))
                    gslot += 1
            if h == 0:
                ch.nA = gslot
        ch.nB = gslot - ch.nA
        tilebase += gslot
        chunks.append(ch)
    assert tilebase == T
    NTmax = max(ch.nA + ch.nB for ch in chunks)

    # gidx column ranges per (chunk, half): laid consecutively
    col = 0
    for ch in chunks:
        ch.gcolA = col
        col += ch.nA * 8
        ch.gcolB = col
        col += ch.nB * 8
    gcols = col
    # rb gather idx columns: 8 per rb, chunk slice = rbs
    for ch in chunks:
        ch.rbcol = ch.rbs[0] * 8

    # per-core edge data
    core_edge = []
    for k in range(NCORES):
        s, rank, rb, half = percore[k]
        gidx = np.zeros((128, gcols), np.int16)
        oh = np.zeros((128, T * 128), np_f8)
        ohT = np.zeros((128, T * 128), np_f8)
        for ch in chunks:
            for h in (0, 1):
                colbase = ch.gcolA if h == 0 else ch.gcolB
                slot0 = 0 if h == 0 else ch.nA
                nslots = ch.nA if h == 0 else ch.nB
                idxs = np.zeros(nslots * 128, np.int64)
                pos = 0
                for r in ch.rbs:
                    m2 = (rb == r) & (half == h)
                    ss = s[m2] - h * HALF
                    rk = rank[m2] - r * RB
                    nt = int(ntile_rbh[r, h])
                    idxs[pos:pos + len(ss)] = ss
                    # one-hot data, global tile index
                    gt0 = ch.tilebase + slot0 + pos // 128
                    e_in = np.arange(len(ss))
                    gpos = pos + e_in           # slot-local position
                    tt = ch.tilebase + slot0 + gpos // 128
                    pp = gpos % 128
                    oh[pp, tt * 128 + rk] = 1.0
                    ohT[rk, tt * 128 + pp] = 1.0
                    pos += nt * 128
                gidx[:, colbase:colbase + nslots * 8] = _wrap_idx(idxs)
        # rb row gathers: rank rows of this core, half-local with dummy 0
        base_rows = k * NSH + np.arange(NRB * RB)
        rbr = np.minimum(base_rows, (k + 1) * NSH - 1)  # clamp pad rows
        if k < 4:
            ra, rbb = rbr, np.zeros_like(rbr)
        else:
            ra, rbb = np.zeros_like(rbr), rbr - HALF
        core_edge.append(dict(
            gidx=gidx, oh=oh, ohT=ohT,
            rbidxA=_wrap_idx(ra), rbidxB=_wrap_idx(rbb),
        ))

    # ---- queries
    e0 = iperm[np.asarray(inputs["edges"][0], np.int64)]
    e1 = iperm[np.asarray(inputs["edges"][1], np.int64)]
    QSH = Q // NCORES
    qgrp_tiles = np.zeros(4, np.int64)
    qcore = []
    for k in range(NCORES):
        sl = slice(k * QSH, (k + 1) * QSH)
        a, b = e0[sl], e1[sl]
        g = (a >= HALF).astype(np.int64) * 2 + (b >= HALF).astype(np.int64)
        qcore.append((a, b, g))
        for gi in range(4):
            cnt = int((g == gi).sum())
            qgrp_tiles[gi] = max(qgrp_tiles[gi], (cnt + 127) // 128)
    QTT = int(qgrp_tiles.sum())
    # subchunks of <=32 tiles per group
    qchunks = []   # (group, tile0_in_out, ntiles)
    tpos = 0
    for gi in range(4):
        nt = int(qgrp_tiles[gi])
        j = 0
        while j < nt:
            step = min(32, nt - j)
            qchunks.append((gi, tpos + j, step))
            j += step
        tpos += nt

    core_q = []
    for k in range(NCORES):
        a, b, g = qcore[k]
        qi0 = np.zeros(QTT * 128, np.int64)
        qi1 = np.zeros(QTT * 128, np.int64)
        qmap = np.full(QTT * 128, -1, np.int64)
        tpos = 0
        for gi in range(4):
            m = g == gi
            cnt = int(m.sum())
            qi0[tpos:tpos + cnt] = a[m] - (gi >> 1) * HALF
            qi1[tpos:tpos + cnt] = b[m] - (gi & 1) * HALF
            qmap[tpos:tpos + cnt] = np.nonzero(m)[0] + k * QSH
            tpos += int(qgrp_tiles[gi]) * 128
        core_q.append(dict(qidx0=_wrap_idx(qi0), qidx1=_wrap_idx(qi1),
                           qmap=qmap))

    return dict(perm=perm, iperm=iperm, chunks=chunks, ntile_rbh=ntile_rbh,
                T=T, NTmax=NTmax, gcols=gcols, core_edge=core_edge,
                qgrp_tiles=qgrp_tiles, qchunks=qchunks, QTT=QTT,
                core_q=core_q)


def _rep(v):
    """[C] -> [128, C] replicated fp32."""
    return np.ascontiguousarray(np.broadcast_to(
        np.asarray(v, np.float32)[None, :], (128, C)))


def _rep16(v):
    """[C] -> [128, C] replicated fp16."""
    return np.ascontiguousarray(np.broadcast_to(
        np.asarray(v, np.float16)[None, :], (128, C)))


def _repbf(v):
    """[C] -> [128, C] replicated bf16."""
    return np.ascontiguousarray(np.broadcast_to(
        np.asarray(v, np.float32).astype(np_bf16)[None, :], (128, C)))


# ------------------------------------------------------------ program build

TUNE = dict(pb_bufs=3, dve_mod=1, dve_thr=0)


def _build_program(S, fast1, bp2val, upto=None):
    """Build the SPMD Bass program. S is the _prep structure.
    upto: truncate after the named stage (HW bisection; output stays zero).

    Sharded design: each core computes h (and s_dst) only for its own NSH
    nodes; fp16 h shards are AllGathered into a Shared h_full that the
    edge-phase gathers read. s_dst per rank block comes straight out of the
    phase-A matmul (W_aug = [W | W@a_dst]) and stays in SBUF."""
    nc = bacc.Bacc("TRN2", target_bir_lowering=False, debug=False,
                   num_devices=NCORES, num_swdge_queues=4)
    T, NTmax, QTT = S["T"], S["NTmax"], S["QTT"]
    chunks, qchunks = S["chunks"], S["qchunks"]

    def din(name, shape, dt):
        return nc.dram_tensor(name, shape, dt, kind="ExternalInput")

    # inputs
    embTk = din("embTk", [128, NSH], BF16)       # per-core node slice
    W1aug = din("W1aug", [128, C + 2], BF16)     # [W1 | W1@a_src1 | W1@a_dst1]
    W2aug = din("W2aug", [128, C + 2], BF16)
    Arep = din("Arep", [128, C], F32)
    C1rep = din("C1rep", [128, C], F32)
    b2rep = din("b2rep", [128, C], F32)
    Wp1bf = din("Wp1bf", [128, C], BF16)
    wp2bf = din("wp2bf", [128, 1], BF16)
    bp1col = din("bp1col", [128, 1], F32)
    ident = din("ident", [128, 128], F32)
    ohA = din("ohA", [128, T * 128], F8)
    ohTA = din("ohTA", [128, T * 128], F8)
    gidx = din("gidx", [128, S["gcols"]], I16)
    qidx0 = din("qidx0", [128, QTT * 8], I16)
    qidx1 = din("qidx1", [128, QTT * 8], I16)

    out_q = nc.dram_tensor("out_q", [1, QTT * 128], F32,
                           kind="ExternalOutput")

    # internal DRAM. h tables carry 512B rows [h(128) | s_src | 1 | pad]:
    # random 256B reads are latency-bound, so the extra 256B is ~free and
    # removes the per-edge s_src dot and the separate denominator matmul.
    CW = 256
    h_sh = nc.dram_tensor("h_sh", [NSH, CW], BF16)
    h_full1 = nc.dram_tensor("h_full1", [N, CW], BF16, addr_space="Shared")
    h_full2 = nc.dram_tensor("h_full2", [N, CW], BF16, addr_space="Shared")
    x1T_sh = nc.dram_tensor("x1T_sh", [128, NSH], BF16)
    x2_sh = nc.dram_tensor("x2_sh", [NSH, C], BF16)
    x2_full = nc.dram_tensor("x2_full", [N, C], BF16, addr_space="Shared")

    with tile.TileContext(nc) as tc:
        with tc.tile_pool(name="const", bufs=1) as cpool:
            def load_const(t, w=C, dt=F32):
                sb = cpool.tile([128, w], dt, tag=t.name)
                nc.sync.dma_start(sb[:], t[:])
                return sb

            W1_sb = load_const(W1aug, w=C + 2, dt=BF16)
            W2_sb = load_const(W2aug, w=C + 2, dt=BF16)
            b2_sb = load_const(b2rep)
            Wp1_sb = load_const(Wp1bf, dt=BF16)
            wp2_sb = load_const(wp2bf, w=1, dt=BF16)
            id_sb = load_const(ident, 128)
            bp1_sb = cpool.tile([128, 1], F32, tag="bp1col")
            nc.sync.dma_start(bp1_sb[:], bp1col[:])
            if not fast1:
                A_sb = load_const(Arep)
                C1_sb = load_const(C1rep)
            gidx_sb = cpool.tile([128, S["gcols"]], I16)
            nc.sync.dma_start(gidx_sb[:], gidx[:])
            q0_sb = cpool.tile([128, QTT * 8], I16)
            nc.sync.dma_start(q0_sb[:], qidx0[:])
            q1_sb = cpool.tile([128, QTT * 8], I16)
            nc.sync.dma_start(q1_sb[:], qidx1[:])
            # s_dst per rank, produced by phase A, consumed by phase B.
            # memset first: phase A leaves pad ranks (NSH % 128 tail) of the
            # last rank block unwritten, and garbage there turns into NaN via
            # 0 * NaN in the s_dst broadcast matmul.
            sdst1_sb = cpool.tile([128, NRB], F32, tag="sdst1")
            nc.gpsimd.memset(sdst1_sb[:], 0.0)
            sdst2_sb = cpool.tile([128, NRB], F32, tag="sdst2")
            nc.gpsimd.memset(sdst2_sb[:], 0.0)

            # --- phase A: h_aug = x @ [W | W@a_src | W@a_dst] for own nodes
            def phase_a(waug_sb, layer, sdst_sb):
                with tc.tile_pool(name="pa", bufs=3) as pa, \
                     tc.tile_pool(name="pap", bufs=2, space="PSUM") as pap:
                    j = 0
                    while j < NSH:
                        w = min(512, NSH - j)
                        xt = pa.tile([128, 512], BF16, tag="pa_xt")
                        if layer == 1:
                            nc.sync.dma_start(xt[:, 0:w], embTk[:, j:j + w])
                        else:
                            nc.sync.dma_start(xt[:, 0:w], x1T_sh[:, j:j + w])
                        nt = (w + 127) // 128
                        for t in range(nt):
                            tw = min(128, w - t * 128)
                            ps = pap.tile([128, C + 2], F32, tag="pa_ps")
                            nc.tensor.matmul(ps[0:tw, :],
                                             xt[:, t * 128:t * 128 + tw],
                                             waug_sb[:], start=True,
                                             stop=True)
                            # table row = [h | s_src | 1 | pad]
                            hs = pa.tile([128, CW], BF16, tag="pa_hs")
                            nc.scalar.activation(hs[0:tw, 0:C + 1],
                                                 ps[0:tw, 0:C + 1], AF.Copy)
                            nc.vector.memset(hs[0:tw, C + 1:C + 2], 1.0)
                            col = j // 128 + t
                            nc.vector.tensor_copy(sdst_sb[0:tw, col:col + 1],
                                                  ps[0:tw, C + 1:C + 2])
                            nc.sync.dma_start(
                                h_sh[j + t * 128:j + t * 128 + tw, :],
                                hs[0:tw, :])
                        j += w

            # ---------------- phase B: edge processing ----------------
            def phase_b(layer, sdst_sb, h_full):
                with tc.tile_pool(name="pb", bufs=TUNE["pb_bufs"]) as pb, \
                     tc.tile_pool(name="pbs", bufs=2) as pbs, \
                     tc.tile_pool(name="pot", bufs=8) as pot, \
                     tc.tile_pool(name="pbp", bufs=2, space="PSUM") as pbp:

                    def emit_front(ch):
                        nt = ch.nA + ch.nB
                        nrb = len(ch.rbs)
                        G = pb.tile([128, NTmax, CW], BF16, tag="G")
                        # 4-way queue split: random 512B reads are
                        # latency-bound, parallel queues drain ~2x faster
                        segs = []
                        for base, cnt, gcol, lo, hi in (
                                (0, ch.nA, ch.gcolA, 0, HALF),
                                (ch.nA, ch.nB, ch.gcolB, HALF, N)):
                            h1 = cnt // 2
                            if h1:
                                segs.append((base, h1, gcol, lo, hi))
                            if cnt - h1:
                                segs.append((base + h1, cnt - h1,
                                             gcol + h1 * 8, lo, hi))
                        for q, (t0, tn, gc, lo, hi) in enumerate(segs):
                            nc.gpsimd.dma_gather(
                                G[:, t0:t0 + tn, :], h_full[lo:hi, :],
                                gidx_sb[:, gc:gc + tn * 8],
                                tn * 128, tn * 128, CW,
                                single_packet=False, queue_num=q % 4)
                        ohT_sb = pb.tile([128, NTmax * 128], F8, tag="ohT")
                        nc.sync.dma_start(
                            ohT_sb[:, 0:nt * 128],
                            ohTA[:, ch.tilebase * 128:(ch.tilebase + nt) * 128])
                        oh_sb = pb.tile([128, NTmax * 128], F8, tag="oh")
                        nc.sync.dma_start(
                            oh_sb[:, 0:nt * 128],
                            ohA[:, ch.tilebase * 128:(ch.tilebase + nt) * 128])
                        # s_dst per rank block: straight from phase A
                        sdb = pbs.tile([128, RBS_PER_CHUNK], BF16, tag="sdb")
                        r0 = ch.rbs[0]
                        nc.vector.tensor_copy(sdb[:, 0:nrb],
                                              sdst_sb[:, r0:r0 + nrb])
                        # s_dst broadcast to edges (PE, N=1 per tile)
                        ps_sd = pbp.tile([128, NTmax], F32, tag="ps_sd")
                        for t, tl in enumerate(ch.tiles):
                            nc.tensor.matmul(
                                ps_sd[:, t:t + 1],
                                ohT_sb[:, t * 128:(t + 1) * 128],
                                sdb[:, tl.rbslot:tl.rbslot + 1],
                                start=True, stop=True, skip_group_check=True)
                        # z = s_src (gathered, table col C) + s_dst
                        z = pbs.tile([128, NTmax], F32, tag="z")
                        nc.vector.tensor_add(z[:, 0:nt], G[:, 0:nt, C],
                                             ps_sd[:, 0:nt])
                        lr = pbs.tile([128, NTmax], F32, tag="lr")
                        # leaky relu on DVE: max(0.2*z, z)
                        nc.vector.scalar_tensor_tensor(
                            lr[:, 0:nt], z[:, 0:nt], 0.2, z[:, 0:nt],
                            OP.mult, OP.max)
                        w_bf = pbs.tile([128, NTmax], F32, tag="w")
                        nc.scalar.activation(w_bf[:, 0:nt], lr[:, 0:nt],
                                             AF.Exp)
                        # aggregation: fold w into the one-hot stationary;
                        # rhs [h | s_src | 1] => col C+1 is the denominator
                        ps_pack = pbp.tile([128, RBS_PER_CHUNK, C + 2], F32,
                                           tag="ps_pack")
                        for t, tl in enumerate(ch.tiles):
                            ohw = pot.tile([128, 128], BF16, tag="ohw")
                            nc.vector.tensor_scalar(
                                ohw[:], oh_sb[:, t * 128:(t + 1) * 128],
                                w_bf[:, t:t + 1], None, OP.mult)
                            nc.tensor.matmul(
                                ps_pack[:, tl.rbslot, :],
                                ohw[:], G[:, t, 0:C + 2],
                                start=tl.first, stop=tl.last,
                                skip_group_check=True)
                        return ps_pack

                    def emit_epi(ch, ps_pack):
                        for i, r in enumerate(ch.rbs):
                            cap = min(RB, NSH - r * RB)
                            dn = pbs.tile([128, 1], F32, tag="dn")
                            nc.vector.tensor_scalar_add(
                                dn[:], ps_pack[:, i, C + 1:C + 2], 1e-16)
                            rcp = pbs.tile([128, 1], F32, tag="rcp")
                            nc.vector.reciprocal(rcp[:], dn[:])
                            if layer == 1:
                                x1b = pbs.tile([128, C], F32, tag="x1b")
                                if fast1:
                                    nc.scalar.activation(
                                        x1b[:], ps_pack[:, i, 0:C], AF.Relu,
                                        scale=rcp[:])
                                else:
                                    y = pbs.tile([128, C], F32, tag="y")
                                    nc.vector.scalar_tensor_tensor(
                                        y[:], ps_pack[:, i, 0:C], rcp[:],
                                        A_sb[:], OP.mult, OP.mult)
                                    y2 = pbs.tile([128, C], F32, tag="y2")
                                    nc.vector.tensor_add(y2[:], y[:],
                                                         C1_sb[:])
                                    nc.scalar.activation(x1b[:], y2[:],
                                                         AF.Relu)
                                ps_t = pbp.tile([128, 128], F32, tag="ps_t")
                                nc.tensor.transpose(ps_t[:], x1b[:], id_sb[:])
                                x1t = pbs.tile([128, 128], BF16, tag="x1t")
                                nc.scalar.activation(x1t[:], ps_t[:], AF.Copy)
                                nc.sync.dma_start(
                                    x1T_sh[:, r * RB:r * RB + cap],
                                    x1t[:, 0:cap])
                            else:
                                x2b = pbs.tile([128, C], BF16, tag="x2b")
                                nc.vector.scalar_tensor_tensor(
                                    x2b[:], ps_pack[:, i, 0:C], rcp[:],
                                    b2_sb[:], OP.mult, OP.add)
                                nc.sync.dma_start(
                                    x2_sh[r * RB:r * RB + cap, :],
                                    x2b[0:cap, :])

                    # software-pipelined: chunk c's epilogue is emitted after
                    # chunk c+1's front so PSUM-read stalls don't head-block
                    # the DVE/Act queues for the next chunk's work.
                    pend = None
                    for ch in chunks:
                        pp = emit_front(ch)
                        if pend is not None:
                            emit_epi(*pend)
                        pend = (ch, pp)
                    emit_epi(*pend)

            # ---------------- run the stages ----------------
            def allgather(src, dst):
                nc.gpsimd.collective_compute(
                    "AllGather", OP.bypass,
                    replica_groups=[list(range(NCORES))],
                    ins=[src.ap().opt()], outs=[dst.ap().opt()])

            stages = ["A1", "AGh1", "B1", "A2", "AGh2", "B2", "AGx2"]
            cut = stages.index(upto) if upto in stages else len(stages)

            if cut >= 0:
                with nc.named_scope("A1"):
                    phase_a(W1_sb, 1, sdst1_sb)
            if cut >= 1:
                with nc.named_scope("AGh1"):
                    allgather(h_sh, h_full1)
            if cut >= 2:
                with nc.named_scope("B1"):
                    phase_b(1, sdst1_sb, h_full1)
            if cut >= 3:
                with nc.named_scope("A2"):
                    phase_a(W2_sb, 2, sdst2_sb)
            if cut >= 4:
                with nc.named_scope("AGh2"):
                    allgather(h_sh, h_full2)
            if cut >= 5:
                with nc.named_scope("B2"):
                    phase_b(2, sdst2_sb, h_full2)
            if cut >= 6:
                with nc.named_scope("AGx2"):
                    allgather(x2_sh, x2_full)

            # ------- phase C: link predictor (transposed, batched) -------
            # gather x2 endpoint features transposed [c, q]; per 512-query
            # block: hq = U*V (DVE), z1 = Wp1^T @ hq + bp1 (PE, bias via a
            # K=1 matmul), relu (Act), out = wp2^T @ z1 (PE), sigmoid (Act).
            qgrp_tiles = S["qgrp_tiles"]
            NQmax = int(max(qgrp_tiles)) * 128
            with nc.named_scope("C"), \
                 tc.tile_pool(name="pc", bufs=2) as pc, \
                 tc.tile_pool(name="pcs", bufs=2) as pcs, \
                 tc.tile_pool(name="pcp", bufs=2, space="PSUM") as pcp:
                t0 = 0
                for gi in range(4 if cut >= 7 else 0):
                    ng = int(qgrp_tiles[gi]) * 128
                    if ng == 0:
                        continue
                    b0 = (gi >> 1) * HALF
                    b1 = (gi & 1) * HALF
                    U = pc.tile([128, 1, NQmax], BF16, tag="U")
                    V = pc.tile([128, 1, NQmax], BF16, tag="V")
                    nc.gpsimd.dma_gather(
                        U[:, :, 0:ng], x2_full[b0:b0 + HALF, :],
                        q0_sb[:, t0 * 8:t0 * 8 + ng // 16],
                        ng, ng, C, transpose=True, single_packet=False)
                    nc.gpsimd.dma_gather(
                        V[:, :, 0:ng], x2_full[b1:b1 + HALF, :],
                        q1_sb[:, t0 * 8:t0 * 8 + ng // 16],
                        ng, ng, C, transpose=True, single_packet=False,
                        queue_num=1)
                    res = pcs.tile([1, NQmax], F32, tag="res")
                    for o in range(0, ng, 512):
                        blk = min(512, ng - o)
                        hq = pcs.tile([128, 512], BF16, tag="hq")
                        nc.vector.tensor_mul(hq[:, 0:blk], U[:, 0, o:o + blk],
                                             V[:, 0, o:o + blk])
                        ps_z = pcp.tile([128, 512], F32, tag="ps_z")
                        nc.tensor.matmul(ps_z[:, 0:blk], Wp1_sb[:],
                                         hq[:, 0:blk],
                                         start=True, stop=True,
                                         skip_group_check=True)
                        zr = pcs.tile([128, 512], BF16, tag="zr")
                        nc.scalar.activation(zr[:, 0:blk], ps_z[:, 0:blk],
                                             AF.Relu, bias=bp1_sb[:])
                        ps_o = pcp.tile([128, 512], F32, tag="ps_o")
                        nc.tensor.matmul(ps_o[0:1, 0:blk], wp2_sb[:],
                                         zr[:, 0:blk],
                                         start=True, stop=True,
                                         skip_group_check=True)
                        nc.scalar.activation(res[:, o:o + blk],
                                             ps_o[0:1, 0:blk],
                                             AF.Sigmoid, bias=float(bp2val))
                    nc.sync.dma_start(out_q[:, t0 * 128:t0 * 128 + ng],
                                      res[:, 0:ng])
                    t0 += int(qgrp_tiles[gi])

    nc.compile()
    return nc


# ------------------------------------------------------------------- kernel

_CACHE = {}
LAST_RESULT = None


def build_all(inputs):
    """Host prep + program build + per-core input maps. Returns
    (nc, in_maps, S) for kernel() and for external bench harnesses."""
    inputs = {k: np.asarray(v) for k, v in inputs.items()}
    S = _prep(inputs)

    gamma = inputs["gamma"].astype(np.float32)
    rvar = inputs["rvar"].astype(np.float32)
    rmean = inputs["rmean"].astype(np.float32)
    beta = inputs["beta"].astype(np.float32)
    b1 = inputs["b1"].astype(np.float32)
    A = gamma / np.sqrt(rvar + BN_EPS)
    C1 = (b1 - rmean) * A + beta
    fast1 = bool(np.allclose(A, 1.0) and np.allclose(C1, 0.0))
    bp2val = float(np.asarray(inputs["bp2"]).reshape(-1)[0])

    nc = _build_program(S, fast1, bp2val)

    perm = S["perm"]
    emb = inputs["embedding"].astype(np.float32)
    embT_p = np.ascontiguousarray(emb[perm].T)

    W1 = inputs["W1"].astype(np.float32)
    W2 = inputs["W2"].astype(np.float32)
    ad1 = inputs["a_dst1"].astype(np.float32)
    ad2 = inputs["a_dst2"].astype(np.float32)
    as1 = inputs["a_src1"].astype(np.float32)
    as2 = inputs["a_src2"].astype(np.float32)
    W1aug = np.ascontiguousarray(np.concatenate(
        [W1, (W1 @ as1)[:, None], (W1 @ ad1)[:, None]],
        axis=1)).astype(np_bf16)
    W2aug = np.ascontiguousarray(np.concatenate(
        [W2, (W2 @ as2)[:, None], (W2 @ ad2)[:, None]],
        axis=1)).astype(np_bf16)

    common = dict(
        W1aug=W1aug, W2aug=W2aug,
        Arep=_rep(A), C1rep=_rep(C1),
        b2rep=_rep(inputs["b2"]),
        Wp1bf=inputs["Wp1"].astype(np_bf16),
        wp2bf=np.ascontiguousarray(inputs["Wp2"].astype(np_bf16)),
        bp1col=np.ascontiguousarray(
            inputs["bp1"].astype(np.float32)[:, None]),
        ident=np.eye(128, dtype=np.float32),
    )

    in_maps = []
    for k in range(NCORES):
        ce, cq = S["core_edge"][k], S["core_q"][k]
        m = dict(common)
        m.update(embTk=np.ascontiguousarray(
                     embT_p[:, k * NSH:(k + 1) * NSH]).astype(np_bf16),
                 ohA=ce["oh"], ohTA=ce["ohT"], gidx=ce["gidx"],
                 qidx0=cq["qidx0"], qidx1=cq["qidx1"])
        in_maps.append(m)

    return nc, in_maps, S


def unpack_output(results, S):
    out = np.zeros(Q, np.float32)
    for k in range(NCORES):
        vals = np.asarray(results[k]["out_q"])      # [1, QTT*128]
        flat = vals.reshape(-1)                     # gather order == qmap idx
        qmap = S["core_q"][k]["qmap"]
        valid = qmap >= 0
        out[qmap[valid]] = flat[valid]
    return out


def kernel(**inputs):
    global LAST_RESULT
    nc, in_maps, S = build_all(inputs)
    res = run_bass_kernel_spmd(nc, in_maps, list(range(NCORES)))
    LAST_RESULT = res
    return unpack_output(res.results, S)

